# revision 5
# baseline (speedup 1.0000x reference)
"""Trainium2 Bass kernel for nn_DenseAttentionOneHead — v2 schedule.

out_b = X_b (W^T (X_b^T X_b)).  Column-split 8 ways (4 slices x 2 batches),
collective-free.  Per core (slice sl):
  S_sl = X^T X[:, sl]          direct form, 8 psum accumulators over 32 chunks
  M_sl = W^T S_sl              [1024, 256]
  out^T[:, sl] = M_sl^T X^T    N=512 streams, LDW shared across n-chunks
Changes vs v1: out phase reoriented (out^T) so the stationary operand is M
(64 LDW instead of 256), fp16 output (host upcasts + transposes), W scaled
by 64 host-side to dodge fp16 subnormals, and a DMA queue program that keeps
the X row stream fed at PE rate while X^T slabs/W trickle behind it in FIFO
order on the same queues.
"""

import numpy as np

import concourse.mybir as mybir
import concourse.tile as tile
from concourse import bacc
from concourse.bass_utils import run_bass_kernel_spmd

F32 = mybir.dt.float32
F16 = mybir.dt.float16
P = 128
D = 1024
B = 2
N = 4096
NCORES = 8
GROUP = 4            # cores per batch
SL = D // GROUP      # 256-column slice per core
NO = D // P          # 8 tiles along D
NCH = N // P         # 32 row chunks
WSCALE = 64.0        # host-side W prescale (fp16 subnormal dodge)

_compiled = None


def _build():
    nc = bacc.Bacc(None, target_bir_lowering=False, debug=False, num_devices=NCORES)

    # xf column-rotated per core (its 256 target columns first), wf row-rotated
    # identically; xt is the plain X^T.  Same program computes every slice.
    xf = nc.dram_tensor("xf", [N, D], F16, kind="ExternalInput")
    xt = nc.dram_tensor("xt", [D, N], F16, kind="ExternalInput")
    wf = nc.dram_tensor("wf", [D, D], F16, kind="ExternalInput")
    o_out = nc.dram_tensor("o_out", [SL, N], F16, kind="ExternalOutput")

    with tile.TileContext(nc) as tc:
        with (
            tc.tile_pool(name="big", bufs=1) as big,
            tc.tile_pool(name="xin", bufs=32) as xin,
            tc.tile_pool(name="stage", bufs=6) as stage,
            tc.tile_pool(name="psum", bufs=8, space="PSUM") as psum,
        ):
            XT = big.tile([P, NO, N], F16, tag="XT")        # X^T [a, n], 8MB
            Wsb = big.tile([P, NO, D], F16, tag="W")        # W   [e, a], 2MB
            Ssb = big.tile([P, NO, SL], F16, tag="Ssb")     # S_sl [e, d']
            Msb = big.tile([P, NO, SL], F16, tag="Msb")     # M_sl [a, d']
            junk = big.tile([P, P], F16, tag="junk")

            # HAM warmup: ~20 throwaway matmuls during the preamble/first-DMA
            # window so the PE clock gate is already 8/8 when chunk 0 lands.
            nc.vector.memset(junk[:], 0)
            jacc = psum.tile([P, 512], F32, tag="acc", name="jacc")[:, :P]
            for _ in range(16):
                nc.tensor.matmul(jacc[:], junk[:], junk[:], start=True, stop=True)

            # ---- S_sl = X^T X[:, sl]: stream chunks, 8 held accumulators.
            # DMA queue programs (FIFO per engine):
            #   sync:   xc evens                    | slabs 0,2,4,6
            #   scalar: xc odds + W one-per-chunk   | slabs 1,3,5,7
            # The S window carries only xf+W (10MB ~ PE-balanced); all of X^T
            # flows during the M/out windows, comfortably ahead of n-chunk
            # consumption (slab j lands ~S_end+2.7(j+1)us, needed S_end+8.6+
            # 3.4j us).
            saccs = [
                psum.tile([P, 512], F32, tag="acc", name=f"sacc_{et}")[:, :SL]
                for et in range(NO)
            ]


            # Tile reorders independent instructions, so FIFO position alone
            # does NOT keep the W/X^T loads out of the S window (they have no
            # input deps and get hoisted to t=0, starving the X row stream).
            # Gate them with real deps: a tiny DVE copy that reads data of
            # known timing and writes into each DMA's destination region makes
            # the DMA wait via the WAW hazard.  W chunks are gated on mid/late
            # X chunks (the PE-bound S window has ~2.5MB of DMA slack, W is
            # 2MB); X^T slabs are gated on the first S drain and then flow
            # during the M/out windows, well ahead of n-chunk consumption.
            # Queue roles (engine queues have head-of-line blocking and ring
            # FIFOs execute transfers in descriptor order, so W and the X^T
            # slabs must not share a queue):
            #   sync:   xc evens | W (gated mid-S) | out-writes
            #   scalar: xc odds  | X^T slabs (gated on the first S drain)
            #   DVE:    all PSUM drains + the tiny gate copies
            xcs = []
            for ch in range(NCH):
                xc = xin.tile([P, D], F16, tag="xc")
                xcs.append(xc)
                if ch == 0:
                    # quarters on alternating queues: the first matmul only
                    # needs cols 0:256, so it starts ~1us sooner
                    for q in range(4):
                        qeng = nc.sync if q % 2 == 0 else nc.scalar
                        qeng.dma_start(
                            xc[:, q * 256:(q + 1) * 256],
                            xf[0:P, q * 256:(q + 1) * 256],
                        )
                else:
                    # odds on scalar: those issues retire by ~t=18, well
                    # before scalar's drain copies are needed (no HOL risk)
                    eng = nc.sync if ch % 2 == 0 else nc.scalar
                    eng.dma_start(xc[:], xf[ch * P:(ch + 1) * P, :])
                for et in range(NO):
                    nc.tensor.matmul(
                        saccs[et][:],
                        xc[:, et * P:(et + 1) * P],
                        xc[:, :SL],
                        start=(ch == 0),
                        stop=(ch == NCH - 1),
                    )

            # slab0 then W, both gated on chunk 29's landing (~t=28; the xf
            # stream is fully issued and nearly fully landed by then, so these
            # ride the post-stream DMA lull).  slab0 goes FIRST: the out-phase
            # start is its critical consumer, while M does not need W until
            # ~12us later — W still lands with >5us of slack.
            nc.vector.tensor_copy(XT[:, 0, 0:1], xcs[29][:, 0:1])
            srcx0 = xt[:, 0:1024]
            nc.sync.dma_start(
                XT[:, :, 0:1024], srcx0.rearrange("(c p) n -> p c n", p=P)
            )
            for wch in range(NO):
                nc.vector.tensor_copy(Wsb[:, wch, 0:1], xcs[29][:, 0:1])
            for wch in range(NO):
                nc.sync.dma_start(Wsb[:, wch, :], wf[wch * P:(wch + 1) * P, :])

            # Drain S on DVE, et0 first: it releases the X^T slab gates.
            # All drains stay on DVE — an ACT copy would sit behind the slab
            # issues in scalar's FIFO (head-of-line) and stall M.
            nc.vector.tensor_copy(Ssb[:, 0, :], saccs[0][:])
            nc.scalar.copy(Ssb[:, 1, :], saccs[1][:])
            # X^T as 4 n-slices of [128, 8, 1024] on scalar, gated on the
            # first S drain (resolves strictly after all xf consumption, so
            # slab descriptors can never starve the X stream).  The n-sliced
            # shape lets the out phase consume n-chunks while later slices
            # are still in flight; 1024-wide slices keep the descriptor
            # count (and so the engine issue cost) manageable.
            nc.vector.tensor_copy(XT[:, 0, 1024:1025], xcs[31][:, 0:1])
            for j in range(2, 4):
                nc.vector.tensor_copy(
                    XT[:, 0, j * 1024:j * 1024 + 1], Ssb[:, 0, j:j + 1]
                )
            for j in range(1, 4):
                srcx = xt[:, j * 1024:(j + 1) * 1024]
                nc.sync.dma_start(
                    XT[:, :, j * 1024:(j + 1) * 1024],
                    srcx.rearrange("(c p) n -> p c n", p=P),
                )
            for et in range(2, NO):
                if et % 2 == 0:
                    nc.vector.tensor_copy(Ssb[:, et, :], saccs[et][:])
                else:
                    nc.scalar.copy(Ssb[:, et, :], saccs[et][:])

            # ---- M_sl = W^T S_sl.  One full PSUM bank per a-tile accumulator
            # (start=True clears the whole bank, so banks can't be shared),
            # e-outer so the ladder consumes W chunks in DMA arrival order.
            maccs = [
                psum.tile([P, 512], F32, tag="acc", name=f"macc_{at}")[:, :SL]
                for at in range(NO)
            ]
            for ech in range(NO):
                for at in range(NO):
                    nc.tensor.matmul(
                        maccs[at][:],
                        Wsb[:, ech, at * P:(at + 1) * P],
                        Ssb[:, ech, :],
                        start=(ech == 0),
                        stop=(ech == NO - 1),
                    )
            for at in range(NO):
                if at % 2 == 0:
                    nc.vector.tensor_copy(Msb[:, at, :], maccs[at][:])
                else:
                    nc.scalar.copy(Msb[:, at, :], maccs[at][:])

            # ---- out^T[sl, n] = M^T X^T: lhsT = M[a_ch, sl_t] (shared across
            # the n-pair), rhs = XT[a_ch, n-chunk].
            for np_ in range(4):
                oaccs = {
                    (slt, k): psum.tile(
                        [P, 512], F32, tag="acc", name=f"oacc_{np_}_{slt}_{k}"
                    )
                    for slt in range(2)
                    for k in range(2)
                }
                for slt in range(2):
                    for ach in range(NO):
                        for k in range(2):
                            nch = 2 * np_ + k
                            nc.tensor.matmul(
                                oaccs[(slt, k)][:],
                                Msb[:, ach, slt * P:(slt + 1) * P],
                                XT[:, ach, nch * 512:(nch + 1) * 512],
                                start=(ach == 0),
                                stop=(ach == NO - 1),
                            )
                for slt in range(2):
                    for k in range(2):
                        nch = 2 * np_ + k
                        ot = stage.tile([P, 512], F16, tag="ot")
                        # last n-pair: alternate engines by k so the final two
                        # drains and writes run in parallel (shorter tail)
                        use_dve = (slt == 0) if np_ < 3 else (k == 0)
                        if use_dve:
                            nc.vector.tensor_copy(ot[:], oaccs[(slt, k)][:])
                        else:
                            nc.scalar.copy(ot[:], oaccs[(slt, k)][:])
                        weng = nc.sync if (np_ < 3 or k == 0) else nc.scalar
                        weng.dma_start(
                            o_out[slt * P:(slt + 1) * P, nch * 512:(nch + 1) * 512],
                            ot[:],
                        )

    nc.finalize()
    return nc


def _get_compiled():
    global _compiled
    if _compiled is None:
        _compiled = _build()
    return _compiled


def kernel(hidden_states, queries, _trace=False, _trace_cores=None):
    x = np.ascontiguousarray(np.asarray(hidden_states, dtype=np.float32))
    w = np.ascontiguousarray(np.asarray(queries, dtype=np.float32))
    assert x.shape == (B, N, D) and w.shape == (D, D)

    nc = _get_compiled()
    w16 = (w * WSCALE).astype(np.float16)
    x16 = [x[b].astype(np.float16) for b in range(B)]
    xt16 = [np.ascontiguousarray(x16[b].T) for b in range(B)]
    in_maps = []
    for c in range(NCORES):
        b, s = c // GROUP, c % GROUP
        in_maps.append(
            {
                "xf": np.ascontiguousarray(np.roll(x16[b], -s * SL, axis=1)),
                "xt": xt16[b],
                "wf": np.ascontiguousarray(np.roll(w16, -s * SL, axis=0)),
            }
        )

    res = run_bass_kernel_spmd(
        nc,
        in_maps,
        core_ids=list(range(NCORES)),
        trace=_trace,
        trace_cores=_trace_cores,
    )

    out = np.empty((B, N, D), dtype=np.float32)
    inv = 1.0 / WSCALE
    for c in range(NCORES):
        b, s = c // GROUP, c % GROUP
        ot = res.results[c]["o_out"].astype(np.float32)
        out[b, :, s * SL:(s + 1) * SL] = ot.T * inv

    if _trace:
        kernel.last_result = res
    return out
